# revision 11
# baseline (speedup 1.0000x reference)
"""Trainium2 Bass kernel for ChunkedGeoSparseLinear (gather-mode sparse linear).

out[n, o] = sum_k x[n, idx[o, k]] * w[o, k] + b[o]
  x: (4096, 4096) f32, idx: (4096, 16) i64, w: (4096, 16) f32, b: (4096,) f32

Strategy (data-parallel over rows, 8 cores):
  - Host: transpose x -> xT, cast to bf16; core d gets slab xT[:, 512d:512d+512].
  - Device: dma_gather (SWDGE) pulls the 65536 tap rows xT[idx[o,k], :] from HBM
    into SBUF tiles of 128 taps x 512 cols. Taps are pre-ordered so that tile
    (G, m) holds tap m of outputs [128G, 128G+128).
  - PE: per output group G, 16 matmuls with *diagonal* lhsT (diag of w[:, m])
    accumulate the K taps into one PSUM tile [128 outputs, 512 rows].
  - ScalarE drains PSUM with the bias add; DMA writes outT slab; host
    transposes back and stacks.
"""

import sys

import numpy as np
import ml_dtypes

for _p in ("/opt/trn_rl_repo", "/opt/pypackages"):
    if _p not in sys.path:
        sys.path.append(_p)

N = 4096
IN_F = 4096
OUT_F = 4096
K = 16
NCORES = 8
NSLAB = N // NCORES           # 512 rows per core
NGRP = OUT_F // 128           # 32 psum groups of 128 outputs
TAPS = OUT_F * K              # 65536
TPC = 1024                    # taps per dma_gather call (SWDGE ring holds 1024
                              # descriptors; >1024-idx gathers crash the device)
CPG = (128 * K) // TPC        # gather calls per psum group (2)

_CACHE = {}


def _build(reps: int = 1):
    """Build + compile the per-core Bass program (SPMD: same program, 8 cores)."""
    import concourse.bacc as bacc
    import concourse.mybir as mybir
    import concourse.tile as tile

    dt = mybir.dt
    nc = bacc.Bacc("TRN2", debug=False, num_devices=NCORES,
                   enable_partition_id=False, num_swdge_queues=4)

    xt = nc.dram_tensor("xt", [IN_F, NSLAB], dt.bfloat16, kind="ExternalInput")
    idxs = nc.dram_tensor("idxs", [128, TAPS // 16], dt.int16, kind="ExternalInput")
    wcol = nc.dram_tensor("wcol", [128, TAPS // 128], dt.bfloat16, kind="ExternalInput")
    bias = nc.dram_tensor("bias", [128, NGRP], dt.float32, kind="ExternalInput")
    ident_d = nc.dram_tensor("ident", [128, 128], dt.bfloat16, kind="ExternalInput")
    outT = nc.dram_tensor("outT", [OUT_F, NSLAB], dt.float32, kind="ExternalOutput")
    # reps-dependent output shape keeps timing variants from aliasing in the
    # executable cache (the cache key ignores the embedded BIR)
    nc.dram_tensor("repstag", [1, reps], dt.float32, kind="ExternalOutput")

    with tile.TileContext(nc) as tc:
        with (
            tc.tile_pool(name="singles", bufs=1) as singles,
            tc.tile_pool(name="gpool", bufs=10) as gpool,
            tc.tile_pool(name="dpool", bufs=3) as dpool,
            tc.tile_pool(name="ppool", bufs=4, space="PSUM") as ppool,
            tc.tile_pool(name="opool", bufs=4) as opool,
        ):
            idxs_sb = singles.tile([128, TAPS // 16], dt.int16)
            nc.sync.dma_start(idxs_sb[:], idxs[:])
            w_sb = singles.tile([128, TAPS // 128], dt.bfloat16)
            nc.sync.dma_start(w_sb[:], wcol[:])
            bias_sb = singles.tile([128, NGRP], dt.float32)
            nc.sync.dma_start(bias_sb[:], bias[:])
            ident = singles.tile([128, 128], dt.bfloat16)
            nc.sync.dma_start(ident[:], ident_d[:])

            def body(_i=None):
                ident_b = ident[:].unsqueeze(1).broadcast_to([128, K, 128])
                tiles_per_call = TPC // 128  # 8
                for G in range(NGRP):
                    gs = []
                    for j in range(CPG):
                        c = G * CPG + j
                        g = gpool.tile([128, tiles_per_call, NSLAB], dt.bfloat16)
                        nc.gpsimd.dma_gather(
                            g[:], xt[:],
                            idxs_sb[:, c * (TPC // 16):(c + 1) * (TPC // 16)],
                            TPC, TPC, NSLAB,
                            queue_num=c % 4,
                        )
                        gs.append(g)
                    diag = dpool.tile([128, K, 128], dt.bfloat16)
                    wb = (w_sb[:, G * K:(G + 1) * K]
                          .unsqueeze(2).broadcast_to([128, K, 128]))
                    nc.vector.tensor_tensor(diag[:], ident_b, wb,
                                            op=mybir.AluOpType.mult)
                    p = ppool.tile([128, NSLAB], dt.float32)
                    for m in range(K):
                        nc.tensor.matmul(
                            p[:], diag[:, m, :],
                            gs[m // tiles_per_call][:, m % tiles_per_call, :],
                            start=(m == 0), stop=(m == K - 1))
                    o = opool.tile([128, NSLAB], dt.float32)
                    nc.scalar.activation(
                        o[:], p[:], mybir.ActivationFunctionType.Identity,
                        bias=bias_sb[:, G:G + 1])
                    nc.sync.dma_start(outT[G * 128:(G + 1) * 128, :], o[:])

            if reps == 1:
                body()
            else:
                with tc.For_i(0, reps, 1):
                    body()

    nc.compile()
    return nc


def _prep_inputs(x, in_index_per_out, weight, bias):
    """Host-side data prep shared by all cores + per-core x slabs."""
    idx = np.asarray(in_index_per_out).astype(np.int64)
    w = np.asarray(weight).astype(np.float32)
    b = np.asarray(bias).astype(np.float32)

    # tap order: flat[(G*16 + m)*128 + p] = idx[128G + p, m]
    idx_flat = idx.reshape(NGRP, 128, K).transpose(0, 2, 1).reshape(-1)
    wrap = idx_flat.reshape(TAPS // 16, 16).T          # [16, TAPS//16]
    idxs_np = np.tile(wrap, (8, 1)).astype(np.int16)   # [128, TAPS//16]

    # wcol[p, G*16 + m] = w[128G + p, m]
    wcol_np = (w.reshape(NGRP, 128, K).transpose(1, 0, 2)
               .reshape(128, NGRP * K).astype(ml_dtypes.bfloat16))
    bias_np = np.ascontiguousarray(b.reshape(NGRP, 128).T)  # [128, NGRP]
    ident_np = np.eye(128, dtype=ml_dtypes.bfloat16)

    xT = np.ascontiguousarray(np.asarray(x).astype(np.float32).T
                              .astype(ml_dtypes.bfloat16))  # (IN_F, N)
    slabs = [np.ascontiguousarray(xT[:, d * NSLAB:(d + 1) * NSLAB])
             for d in range(NCORES)]
    return idxs_np, wcol_np, bias_np, ident_np, slabs


def kernel(x, in_index_per_out, weight, bias):
    from concourse import bass_utils

    idxs_np, wcol_np, bias_np, ident_np, slabs = _prep_inputs(
        x, in_index_per_out, weight, bias)

    if "nc" not in _CACHE:
        _CACHE["nc"] = _build(reps=1)
    nc = _CACHE["nc"]

    in_maps = [
        {"xt": slabs[d], "idxs": idxs_np, "wcol": wcol_np,
         "bias": bias_np, "ident": ident_np}
        for d in range(NCORES)
    ]
    res = bass_utils.run_bass_kernel_spmd(nc, in_maps,
                                          core_ids=list(range(NCORES)))
    out = np.empty((N, OUT_F), dtype=np.float32)
    for d in range(NCORES):
        out[d * NSLAB:(d + 1) * NSLAB, :] = res.results[d]["outT"].T
    return out


# revision 12
# speedup vs baseline: 1.1136x; 1.1136x over previous
"""Trainium2 Bass kernel for ChunkedGeoSparseLinear (gather-mode sparse linear).

out[n, o] = sum_k x[n, idx[o, k]] * w[o, k] + b[o]
  x: (4096, 4096) f32, idx: (4096, 16) i64, w: (4096, 16) f32, b: (4096,) f32

Strategy (data-parallel over rows, 8 cores):
  - Host: transpose x -> xT, cast to bf16; core d gets slab xT[:, 512d:512d+512].
  - Device: dma_gather (SWDGE) pulls the 65536 tap rows xT[idx[o,k], :] from HBM
    into SBUF tiles of 128 taps x 512 cols. Taps are pre-ordered so that tile
    (G, m) holds tap m of outputs [128G, 128G+128).
  - PE: per output group G, 16 matmuls with *diagonal* lhsT (diag of w[:, m])
    accumulate the K taps into one PSUM tile [128 outputs, 512 rows].
  - ScalarE drains PSUM with the bias add; DMA writes outT slab; host
    transposes back and stacks.
"""

import sys

import numpy as np
import ml_dtypes

for _p in ("/opt/trn_rl_repo", "/opt/pypackages"):
    if _p not in sys.path:
        sys.path.append(_p)

N = 4096
IN_F = 4096
OUT_F = 4096
K = 16
NCORES = 8
NSLAB = N // NCORES           # 512 rows per core
NGRP = OUT_F // 128           # 32 psum groups of 128 outputs
TAPS = OUT_F * K              # 65536
TPC = 512                     # taps per dma_gather call (SWDGE ring holds 1024
                              # descriptors; >1024-idx gathers crash the device)
CPG = (128 * K) // TPC        # gather calls per psum group (2)

_CACHE = {}


def _build(reps: int = 1):
    """Build + compile the per-core Bass program (SPMD: same program, 8 cores)."""
    import concourse.bacc as bacc
    import concourse.mybir as mybir
    import concourse.tile as tile

    dt = mybir.dt
    nc = bacc.Bacc("TRN2", debug=False, num_devices=NCORES,
                   enable_partition_id=False, num_swdge_queues=4)

    xt = nc.dram_tensor("xt", [IN_F, NSLAB], dt.bfloat16, kind="ExternalInput")
    idxs = nc.dram_tensor("idxs", [128, TAPS // 16], dt.int16, kind="ExternalInput")
    wcol = nc.dram_tensor("wcol", [128, TAPS // 128], dt.bfloat16, kind="ExternalInput")
    bias = nc.dram_tensor("bias", [128, NGRP], dt.float32, kind="ExternalInput")
    ident_d = nc.dram_tensor("ident", [128, 128], dt.bfloat16, kind="ExternalInput")
    outT = nc.dram_tensor("outT", [OUT_F, NSLAB], dt.float32, kind="ExternalOutput")
    # reps-dependent output shape keeps timing variants from aliasing in the
    # executable cache (the cache key ignores the embedded BIR)
    nc.dram_tensor("repstag", [1, reps], dt.float32, kind="ExternalOutput")

    with tile.TileContext(nc) as tc:
        with (
            tc.tile_pool(name="singles", bufs=1) as singles,
            tc.tile_pool(name="gpool", bufs=20) as gpool,
            tc.tile_pool(name="dpool", bufs=3) as dpool,
            tc.tile_pool(name="ppool", bufs=4, space="PSUM") as ppool,
            tc.tile_pool(name="opool", bufs=4) as opool,
        ):
            idxs_sb = singles.tile([128, TAPS // 16], dt.int16)
            nc.sync.dma_start(idxs_sb[:], idxs[:])
            w_sb = singles.tile([128, TAPS // 128], dt.bfloat16)
            nc.sync.dma_start(w_sb[:], wcol[:])
            bias_sb = singles.tile([128, NGRP], dt.float32)
            nc.sync.dma_start(bias_sb[:], bias[:])
            ident = singles.tile([128, 128], dt.bfloat16)
            nc.sync.dma_start(ident[:], ident_d[:])

            def body(_i=None):
                ident_b = ident[:].unsqueeze(1).broadcast_to([128, K, 128])
                tiles_per_call = TPC // 128
                for G in range(NGRP):
                    gs = []
                    for j in range(CPG):
                        c = G * CPG + j
                        g = gpool.tile([128, tiles_per_call, NSLAB], dt.bfloat16)
                        nc.gpsimd.dma_gather(
                            g[:], xt[:],
                            idxs_sb[:, c * (TPC // 16):(c + 1) * (TPC // 16)],
                            TPC, TPC, NSLAB,
                            queue_num=c % 4,
                        )
                        gs.append(g)
                    diag = dpool.tile([128, K, 128], dt.bfloat16)
                    wb = (w_sb[:, G * K:(G + 1) * K]
                          .unsqueeze(2).broadcast_to([128, K, 128]))
                    nc.vector.tensor_tensor(diag[:], ident_b, wb,
                                            op=mybir.AluOpType.mult)
                    p = ppool.tile([128, NSLAB], dt.float32)
                    for m in range(K):
                        nc.tensor.matmul(
                            p[:], diag[:, m, :],
                            gs[m // tiles_per_call][:, m % tiles_per_call, :],
                            start=(m == 0), stop=(m == K - 1))
                    o = opool.tile([128, NSLAB], dt.float32)
                    nc.scalar.activation(
                        o[:], p[:], mybir.ActivationFunctionType.Identity,
                        bias=bias_sb[:, G:G + 1])
                    nc.sync.dma_start(outT[G * 128:(G + 1) * 128, :], o[:])

            if reps == 1:
                body()
            else:
                with tc.For_i(0, reps, 1):
                    body()

    nc.compile()
    return nc


def _prep_inputs(x, in_index_per_out, weight, bias):
    """Host-side data prep shared by all cores + per-core x slabs."""
    idx = np.asarray(in_index_per_out).astype(np.int64)
    w = np.asarray(weight).astype(np.float32)
    b = np.asarray(bias).astype(np.float32)

    # tap order: flat[(G*16 + m)*128 + p] = idx[128G + p, m]
    idx_flat = idx.reshape(NGRP, 128, K).transpose(0, 2, 1).reshape(-1)
    wrap = idx_flat.reshape(TAPS // 16, 16).T          # [16, TAPS//16]
    idxs_np = np.tile(wrap, (8, 1)).astype(np.int16)   # [128, TAPS//16]

    # wcol[p, G*16 + m] = w[128G + p, m]
    wcol_np = (w.reshape(NGRP, 128, K).transpose(1, 0, 2)
               .reshape(128, NGRP * K).astype(ml_dtypes.bfloat16))
    bias_np = np.ascontiguousarray(b.reshape(NGRP, 128).T)  # [128, NGRP]
    ident_np = np.eye(128, dtype=ml_dtypes.bfloat16)

    xT = np.ascontiguousarray(np.asarray(x).astype(np.float32).T
                              .astype(ml_dtypes.bfloat16))  # (IN_F, N)
    slabs = [np.ascontiguousarray(xT[:, d * NSLAB:(d + 1) * NSLAB])
             for d in range(NCORES)]
    return idxs_np, wcol_np, bias_np, ident_np, slabs


def kernel(x, in_index_per_out, weight, bias):
    from concourse import bass_utils

    idxs_np, wcol_np, bias_np, ident_np, slabs = _prep_inputs(
        x, in_index_per_out, weight, bias)

    if "nc" not in _CACHE:
        _CACHE["nc"] = _build(reps=1)
    nc = _CACHE["nc"]

    in_maps = [
        {"xt": slabs[d], "idxs": idxs_np, "wcol": wcol_np,
         "bias": bias_np, "ident": ident_np}
        for d in range(NCORES)
    ]
    res = bass_utils.run_bass_kernel_spmd(nc, in_maps,
                                          core_ids=list(range(NCORES)))
    out = np.empty((N, OUT_F), dtype=np.float32)
    for d in range(NCORES):
        out[d * NSLAB:(d + 1) * NSLAB, :] = res.results[d]["outT"].T
    return out


# revision 13
# speedup vs baseline: 1.3506x; 1.2128x over previous
"""Trainium2 Bass kernel for ChunkedGeoSparseLinear (gather-mode sparse linear).

out[n, o] = sum_k x[n, idx[o, k]] * w[o, k] + b[o]
  x: (4096, 4096) f32, idx: (4096, 16) i64, w: (4096, 16) f32, b: (4096,) f32

Strategy (data-parallel over rows, 8 cores):
  - Host: transpose x -> xT, cast to bf16; core d gets slab xT[:, 512d:512d+512].
  - Device: dma_gather (SWDGE) pulls the 65536 tap rows xT[idx[o,k], :] from HBM
    into SBUF tiles of 128 taps x 512 cols. Taps are pre-ordered so that tile
    (G, m) holds tap m of outputs [128G, 128G+128).
  - PE: per output group G, 16 matmuls with *diagonal* lhsT (diag of w[:, m])
    accumulate the K taps into one PSUM tile [128 outputs, 512 rows].
  - ScalarE drains PSUM with the bias add; DMA writes outT slab; host
    transposes back and stacks.
"""

import sys

import numpy as np
import ml_dtypes

for _p in ("/opt/trn_rl_repo", "/opt/pypackages"):
    if _p not in sys.path:
        sys.path.append(_p)

N = 4096
IN_F = 4096
OUT_F = 4096
K = 16
NCORES = 8
NSLAB = N // NCORES           # 512 rows per core
NGRP = OUT_F // 128           # 32 psum groups of 128 outputs
TAPS = OUT_F * K              # 65536
TPC = 512                     # taps per dma_gather call (SWDGE ring holds 1024
                              # descriptors; >1024-idx gathers crash the device)
CPG = (128 * K) // TPC        # gather calls per psum group (2)

_CACHE = {}


def _build(reps: int = 1):
    """Build + compile the per-core Bass program (SPMD: same program, 8 cores)."""
    import concourse.bacc as bacc
    import concourse.mybir as mybir
    import concourse.tile as tile

    dt = mybir.dt
    nc = bacc.Bacc("TRN2", debug=False, num_devices=NCORES,
                   enable_partition_id=False, num_swdge_queues=4)

    xt = nc.dram_tensor("xt", [IN_F, NSLAB], dt.bfloat16, kind="ExternalInput")
    idxs = nc.dram_tensor("idxs", [128, TAPS // 16], dt.int16, kind="ExternalInput")
    wcol = nc.dram_tensor("wcol", [128, TAPS // 128], dt.bfloat16, kind="ExternalInput")
    bias = nc.dram_tensor("bias", [128, NGRP], dt.float32, kind="ExternalInput")
    ident_d = nc.dram_tensor("ident", [128, 128], dt.bfloat16, kind="ExternalInput")
    outT = nc.dram_tensor("outT", [OUT_F, NSLAB], dt.float32, kind="ExternalOutput")
    # reps-dependent output shape keeps timing variants from aliasing in the
    # executable cache (the cache key ignores the embedded BIR)
    nc.dram_tensor("repstag", [1, reps], dt.float32, kind="ExternalOutput")

    with tile.TileContext(nc) as tc:
        with (
            tc.tile_pool(name="singles", bufs=1) as singles,
            tc.tile_pool(name="gpool", bufs=24) as gpool,
            tc.tile_pool(name="dpool", bufs=4) as dpool,
            tc.tile_pool(name="ppool", bufs=6, space="PSUM") as ppool,
            tc.tile_pool(name="opool", bufs=6) as opool,
        ):
            idxs_sb = singles.tile([128, TAPS // 16], dt.int16)
            nc.sync.dma_start(idxs_sb[:], idxs[:])
            w_sb = singles.tile([128, TAPS // 128], dt.bfloat16)
            nc.sync.dma_start(w_sb[:], wcol[:])
            bias_sb = singles.tile([128, NGRP], dt.float32)
            nc.sync.dma_start(bias_sb[:], bias[:])
            ident = singles.tile([128, 128], dt.bfloat16)
            nc.sync.dma_start(ident[:], ident_d[:])

            def body(_i=None):
                ident_b = ident[:].unsqueeze(1).broadcast_to([128, K, 128])
                tiles_per_call = TPC // 128
                for G in range(NGRP):
                    gs = []
                    for j in range(CPG):
                        c = G * CPG + j
                        g = gpool.tile([128, tiles_per_call, NSLAB], dt.bfloat16)
                        nc.gpsimd.dma_gather(
                            g[:], xt[:],
                            idxs_sb[:, c * (TPC // 16):(c + 1) * (TPC // 16)],
                            TPC, TPC, NSLAB,
                            queue_num=c % 4,
                        )
                        gs.append(g)
                    diag = dpool.tile([128, K, 128], dt.bfloat16)
                    wb = (w_sb[:, G * K:(G + 1) * K]
                          .unsqueeze(2).broadcast_to([128, K, 128]))
                    nc.vector.tensor_tensor(diag[:], ident_b, wb,
                                            op=mybir.AluOpType.mult)
                    p = ppool.tile([128, NSLAB], dt.float32)
                    for m in range(K):
                        nc.tensor.matmul(
                            p[:], diag[:, m, :],
                            gs[m // tiles_per_call][:, m % tiles_per_call, :],
                            start=(m == 0), stop=(m == K - 1))
                    o = opool.tile([128, NSLAB], dt.float32)
                    nc.scalar.activation(
                        o[:], p[:], mybir.ActivationFunctionType.Identity,
                        bias=bias_sb[:, G:G + 1])
                    nc.sync.dma_start(outT[G * 128:(G + 1) * 128, :], o[:])

            if reps == 1:
                body()
            else:
                with tc.For_i(0, reps, 1):
                    body()

    nc.compile()
    return nc


def _prep_inputs(x, in_index_per_out, weight, bias):
    """Host-side data prep shared by all cores + per-core x slabs."""
    idx = np.asarray(in_index_per_out).astype(np.int64)
    w = np.asarray(weight).astype(np.float32)
    b = np.asarray(bias).astype(np.float32)

    # tap order: flat[(G*16 + m)*128 + p] = idx[128G + p, m]
    idx_flat = idx.reshape(NGRP, 128, K).transpose(0, 2, 1).reshape(-1)
    wrap = idx_flat.reshape(TAPS // 16, 16).T          # [16, TAPS//16]
    idxs_np = np.tile(wrap, (8, 1)).astype(np.int16)   # [128, TAPS//16]

    # wcol[p, G*16 + m] = w[128G + p, m]
    wcol_np = (w.reshape(NGRP, 128, K).transpose(1, 0, 2)
               .reshape(128, NGRP * K).astype(ml_dtypes.bfloat16))
    bias_np = np.ascontiguousarray(b.reshape(NGRP, 128).T)  # [128, NGRP]
    ident_np = np.eye(128, dtype=ml_dtypes.bfloat16)

    xT = np.ascontiguousarray(np.asarray(x).astype(np.float32).T
                              .astype(ml_dtypes.bfloat16))  # (IN_F, N)
    slabs = [np.ascontiguousarray(xT[:, d * NSLAB:(d + 1) * NSLAB])
             for d in range(NCORES)]
    return idxs_np, wcol_np, bias_np, ident_np, slabs


def kernel(x, in_index_per_out, weight, bias):
    from concourse import bass_utils

    idxs_np, wcol_np, bias_np, ident_np, slabs = _prep_inputs(
        x, in_index_per_out, weight, bias)

    if "nc" not in _CACHE:
        _CACHE["nc"] = _build(reps=1)
    nc = _CACHE["nc"]

    in_maps = [
        {"xt": slabs[d], "idxs": idxs_np, "wcol": wcol_np,
         "bias": bias_np, "ident": ident_np}
        for d in range(NCORES)
    ]
    res = bass_utils.run_bass_kernel_spmd(nc, in_maps,
                                          core_ids=list(range(NCORES)))
    out = np.empty((N, OUT_F), dtype=np.float32)
    for d in range(NCORES):
        out[d * NSLAB:(d + 1) * NSLAB, :] = res.results[d]["outT"].T
    return out
